# revision 1
# baseline (speedup 1.0000x reference)
"""Trainium2 Bass kernel for ChannelLinearAttention.

Math (per batch element, V = queries.reshape(L, HE), all from the raw values):
    G      = V^T V                      [HE, HE]   (Gram over L)
    colsq  = diag(G);  r = 1/sqrt(colsq)
    vs     = sum_l V[l, :]              [HE]
    c      = (vs * r + eps) * r         [HE]
    W      = gamma * G * (r x r)        [HE, HE]
    part   = V @ W + gamma * vs         [L, HE]
    den    = HE + V @ c                 [L]
    out    = V + part / den[:, None]

Sharding: pure data parallel — B=16 batch elements, 2 per NeuronCore on 8 cores.
Matmuls run in bf16 (fp32 PSUM accumulation); the residual add of `queries`
stays in fp32, so global rel err ~1e-5.
"""

import numpy as np
from contextlib import ExitStack

import concourse.bass as bass
import concourse.tile as tile
from concourse import mybir
from concourse.bass_utils import run_bass_kernel_spmd
from concourse.masks import make_identity

FP32 = mybir.dt.float32
BF16 = mybir.dt.bfloat16
AF = mybir.ActivationFunctionType
ALU = mybir.AluOpType
AX = mybir.AxisListType


class _TC(tile.TileContext):
    """TileContext whose tail drain splits its semaphore waits.

    The walrus CoreV3 codegen on this toolchain rejects a CTRL/NOP-class
    instruction with more than 2 sync waits ("Too many sync wait commands").
    Tile's kernel-tail drain aggregates one wait per live semaphore, which
    exceeds that as soon as a kernel touches >2 queues. Split the waits over
    a chain of SP nops (same engine, in order, before the end barrier) so
    each instruction carries at most 2.
    """

    _MAX_WAITS = 1

    def _drain_and_barrier(self, tick_clock, wait_clock):
        from concourse.vector_clock import ScopedClock

        drain_inst = self.nc.sync.drain()
        wait_clock.add_sem_waits(
            drain_inst.ins, ScopedClock({None: tick_clock.global_clock})
        )
        si = drain_inst.ins.sync_info
        if si is not None and si.on_wait and len(si.on_wait) > self._MAX_WAITS:
            waits = list(si.on_wait)
            chunks = [waits[i:i + self._MAX_WAITS]
                      for i in range(0, len(waits), self._MAX_WAITS)]
            si.on_wait.clear()
            si.on_wait.extend(chunks[0])
            for ch in chunks[1:]:
                nop = self.nc.sync.nop(nofuse=True, hint="tail_drain_split")
                if nop.ins.sync_info is None:
                    nop.ins.sync_info = mybir.SyncInfo(on_wait=[], on_update=[])
                nop.ins.sync_info.on_wait.extend(ch)

        self.nc.all_engine_barrier()
        assert self.sems is not None
        popped = self.nc._tile_sem_poison_stack.pop()
        assert popped is self._sem_poison
        self.nc.clear_and_free_semaphores(list(self.sems.allocated().values()))
        self.nc.all_engine_barrier()

P = 128
B, L_FULL, H, E = 16, 4096, 8, 64
HE = H * E            # 512
N_CORES = 8
B_PER = B // N_CORES  # 2
EPS = 1e-6


def _split_sync_waits(nc, max_waits=1):
    """Walrus on this toolchain rejects instructions with more than one sync
    wait ("Too many sync wait commands"). Move extra waits onto preceding
    same-engine nops — the engine executes them in order, so semantics are
    preserved."""
    n = 0
    for f in nc.m.functions:
        for blk in f.blocks:
            new_insts = []
            for inst in blk.instructions:
                si = inst.sync_info
                waits = list(si.on_wait) if (si and si.on_wait) else []
                if len(waits) > max_waits:
                    extra, keep = waits[:-max_waits], waits[-max_waits:]
                    for i in range(0, len(extra), max_waits):
                        nop = mybir.InstNoOp(
                            name=f"I-waitsplit-{n}",
                            sync_info=mybir.SyncInfo(
                                on_wait=list(extra[i:i + max_waits]),
                                on_update=[]),
                            bass_nofuse=True,
                            engine=inst.engine,
                        )
                        n += 1
                        nc.register_instruction(nop, overwrite=True)
                        new_insts.append(nop)
                    si.on_wait.clear()
                    si.on_wait.extend(keep)
                new_insts.append(inst)
            blk.instructions[:] = new_insts


ALL_STAGES = frozenset({"ph1", "tail", "diag", "ph4a", "ph4b"})

# engine placement knobs (tuned via TimelineSim)
TUNE = {
    "vt_copy": "alt",      # scalar | vector | alt (pr=0 DVE, pr=1 ACT)
    "vs_copy": "vector",   # scalar | vector
    "ep_mul": "scalar",    # scalar (ACT Copy*scale) | vector (stt fused mul+add)
    "ep_add": "gpsimd",    # gpsimd | vector   (ignored if ep_mul == vector)
    "ep_dve_mod": 0,       # chunks with i%4 < this go fused-DVE; rest ACT+Pool

    "den_mode": "pemm",     # dve (stt rowsum) | pemm (N=1 matmuls beside part MMs)
    "vq_extra": 3,         # extra v/vb quad ring slots (cross-batch overlap)
    "cast": "gpsimd",      # scalar | gpsimd | alt (fp32->bf16 quad casts)
    "ep_add_dve_mod": 4,   # ep-adds with i%4 < this go to DVE instead of Pool
}


def build_program(b_per=B_PER, L=L_FULL, num_devices=N_CORES, stages=ALL_STAGES,
                  repeat=1):
    NLT = L // P   # number of 128-row l-chunks
    NJ = HE // P   # 4 n-chunks

    nc = bass.Bass("TRN2", target_bir_lowering=False, debug=False,
                   num_devices=num_devices)
    q_d = nc.dram_tensor("q", [b_per, L, HE], FP32, kind="ExternalInput").ap()
    gam_d = nc.dram_tensor("gamma", [1, 1], FP32, kind="ExternalInput").ap()
    out_d = nc.dram_tensor("out", [b_per, L, HE], FP32, kind="ExternalOutput").ap()

    with _TC(nc) as tc, ExitStack() as ctx:
        _build(ctx, tc, out_d, q_d, gam_d, b_per, L, stages, repeat)
    _split_sync_waits(nc)
    return nc


def _build(ctx, tc, out_d, q_d, gam_d, b_per, L, stages=ALL_STAGES, repeat=1):
    nc = tc.nc
    NLT = L // P
    NJ = HE // P

    const = ctx.enter_context(tc.tile_pool(name="const", bufs=1))
    vpool = ctx.enter_context(tc.tile_pool(name="vpool", bufs=NLT // 4 + TUNE["vq_extra"]))
    vbpool = ctx.enter_context(tc.tile_pool(name="vbpool", bufs=NLT // 4 + TUNE["vq_extra"]))
    big = ctx.enter_context(tc.tile_pool(name="big", bufs=1))
    small = ctx.enter_context(tc.tile_pool(name="small", bufs=1))
    scr = ctx.enter_context(tc.tile_pool(name="scr", bufs=2))
    outp = ctx.enter_context(tc.tile_pool(name="outp", bufs=2))
    gps = ctx.enter_context(tc.tile_pool(name="gps", bufs=1, space="PSUM"))
    tp_bufs = 3 if TUNE["den_mode"] == "pemm" else 4
    tps = ctx.enter_context(tc.tile_pool(name="tps", bufs=tp_bufs, space="PSUM"))

    # ---------------- constants ----------------
    ident129 = const.tile([P, P + 1], BF16)    # [I_128 | ones] for transpose+colsum
    make_identity(nc, ident129[:, 0:P])
    nc.gpsimd.memset(ident129[:, P:P + 1], 1.0)
    i128b = const.tile([P, P], BF16)           # identity (diag masks)
    make_identity(nc, i128b)
    ones_kb = const.tile([P, P], BF16)         # all-ones, column-sum matmuls
    nc.gpsimd.memset(ones_kb, 1.0)
    ones_c1 = const.tile([P, 1], BF16)         # ones column, M=1 column-sum matmul
    nc.gpsimd.memset(ones_c1, 1.0)
    ones_r1b = const.tile([1, P], BF16)        # ones row, K=1 matmul lhsT
    nc.gpsimd.memset(ones_r1b, 1.0)
    ones_r1f = const.tile([1, P], FP32)
    nc.gpsimd.memset(ones_r1f, 1.0)
    one_11b = const.tile([1, 1], BF16)
    nc.gpsimd.memset(one_11b, 1.0)

    gam_sb = const.tile([1, 1], FP32)
    nc.sync.dma_start(out=gam_sb, in_=gam_d[:, :])
    # broadcast gamma to all 128 partitions: [1,128]^T @ [1,1]
    gam_ps = tps.tile([P, 1], FP32, tag="tp")
    nc.tensor.matmul(gam_ps, lhsT=ones_r1f, rhs=gam_sb, start=True, stop=True)
    gam_part = const.tile([P, 1], FP32)
    nc.scalar.copy(out=gam_part, in_=gam_ps)

    first = True
    for b in [bb for _ in range(repeat) for bb in range(b_per)]:
        first_quad, first = first, False
        # ------------- phase 1: load, cast, Gram, transpose -------------
        vt_all = big.tile([P, NJ, L], BF16, tag="vt_all")    # V^T, vt[p,j,l] = V[l, 128j+p]
        w_all = big.tile([P, NJ, HE], BF16, tag="w_all")
        vs_cols = small.tile([P, NJ, NLT], FP32, tag="vs_cols")
        # G symmetry: block-row j only needs columns >= 128j
        g_tiles = [gps.tile([P, HE - j * P], FP32, tag=f"g{j}", bufs=1,
                            name=f"g_{b}_{j}")
                   for j in range(NJ)]

        NQ = NLT // 4
        v_tiles, vb_tiles = [], []
        for kq in range(NQ):
            # 1 MB quad load: rows l = 512*kq + 128*s + p
            vq = vpool.tile([P, 4, HE], FP32, tag="v", name=f"v_{b}_{kq}")
            vbq = vbpool.tile([P, 4, HE], BF16, tag="vb", name=f"vb_{b}_{kq}")
            cast_eng = (nc.gpsimd if (TUNE["cast"] == "gpsimd" or
                                      (TUNE["cast"] == "alt" and kq % 2 == 0))
                        else None)
            if first_quad and kq == 0:
                # split the very first load/cast so PE can start ~6us earlier
                for s in range(4):
                    nc.sync.dma_start(
                        out=vq[:, s, :],
                        in_=q_d[b, (4 * kq + s) * P:(4 * kq + s + 1) * P, :])
                    if cast_eng is not None:
                        cast_eng.tensor_copy(out=vbq[:, s, :], in_=vq[:, s, :])
                    else:
                        nc.scalar.copy(out=vbq[:, s, :], in_=vq[:, s, :])
            else:
                nc.sync.dma_start(
                    out=vq,
                    in_=q_d[b, kq * 4 * P:(kq + 1) * 4 * P, :].rearrange(
                        "(s p) n -> p s n", p=P))
                if cast_eng is not None:
                    cast_eng.tensor_copy(out=vbq, in_=vq)
                else:
                    nc.scalar.copy(out=vbq, in_=vq)
            for s in range(4):
                v_tiles.append(vq[:, s, :])
                vb_tiles.append(vbq[:, s, :])
        if "ph1" in stages:
            for k in range(NLT):
                vb = vb_tiles[k]
                # Gram, upper triangle: G[128j+m, n>=128j] += V[l,128j+m] V[l,n]
                for j in range(NJ):
                    nc.tensor.matmul(g_tiles[j], lhsT=vb[:, j * P:(j + 1) * P],
                                     rhs=vb[:, j * P:], start=(k == 0),
                                     stop=(k == NLT - 1))
                # transpose blocks (plus ones-column => per-tile column sums)
                for pr in range(NJ // 2):
                    t = tps.tile([P, 2, P + 1], FP32, tag="tp",
                                 name=f"t_{b}_{k}_{pr}")
                    for jj in range(2):
                        j = 2 * pr + jj
                        nc.tensor.matmul(t[:, jj, :],
                                         lhsT=vb[:, j * P:(j + 1) * P],
                                         rhs=ident129, start=True, stop=True)
                    mode = TUNE["vt_copy"]
                    use_act = (mode == "scalar" or
                               (mode == "alt" and pr == 1))
                    if use_act:
                        nc.scalar.copy(
                            out=vt_all[:, 2 * pr:2 * pr + 2, k * P:(k + 1) * P],
                            in_=t[:, :, 0:P])
                    else:
                        nc.vector.tensor_copy(
                            out=vt_all[:, 2 * pr:2 * pr + 2, k * P:(k + 1) * P],
                            in_=t[:, :, 0:P])
                    vs_eng = nc.scalar if TUNE["vs_copy"] == "scalar" else nc.vector
                    if TUNE["vs_copy"] == "scalar":
                        vs_eng.copy(out=vs_cols[:, 2 * pr:2 * pr + 2, k:k + 1],
                                    in_=t[:, :, P:P + 1])
                    else:
                        vs_eng.tensor_copy(
                            out=vs_cols[:, 2 * pr:2 * pr + 2, k:k + 1],
                            in_=t[:, :, P:P + 1])

        # ------------- phase 2/3: tail math -------------
        if "tail" not in stages:
            nc.sync.dma_start(out=out_d[b, 0:P, :], in_=v_tiles[0])
            continue
        # vs[128j+p] = sum over l-tiles of the transpose ones-columns
        vs4 = small.tile([P, NJ], FP32, tag="vs4")
        for j in range(NJ):
            nc.vector.tensor_reduce(out=vs4[:, j:j + 1], in_=vs_cols[:, j, :],
                                    axis=AX.X, op=ALU.add)
        # colsq[128j+p] = G[128j+p, 128j+p]: masked row-sum of G's diag block
        colsq4 = small.tile([P, NJ], FP32, tag="colsq4")
        dscr = scr.tile([P, NJ, P], FP32, tag="dscr", name=f"dscr_{b}")
        for j in range(NJ):
            nc.vector.scalar_tensor_tensor(out=dscr[:, j, :],
                                           in0=g_tiles[j][:, 0:P],
                                           scalar=1.0, in1=i128b,
                                           op0=ALU.mult, op1=ALU.mult,
                                           accum_out=colsq4[:, j:j + 1])
        norm4 = small.tile([P, NJ], FP32, tag="norm4")
        nc.scalar.sqrt(out=norm4, in_=colsq4)
        r4 = small.tile([P, NJ], FP32, tag="r4")
        nc.vector.reciprocal(out=r4, in_=norm4)
        # c = (vs*r + eps) * r
        c4 = small.tile([P, NJ], FP32, tag="c4")
        nc.vector.tensor_mul(out=c4, in0=vs4, in1=r4)
        nc.vector.tensor_scalar(out=c4, in0=c4, scalar1=EPS, scalar2=None,
                                op0=ALU.add)
        nc.vector.tensor_mul(out=c4, in0=c4, in1=r4)
        c4b = small.tile([P, NJ], BF16, tag="c4b")
        nc.vector.tensor_copy(out=c4b, in_=c4)
        # sc4 = gamma * r  (per-partition scale for W rows)
        sc4 = small.tile([P, NJ], FP32, tag="sc4")
        nc.vector.tensor_scalar(out=sc4, in0=r4, scalar1=gam_part, scalar2=None,
                                op0=ALU.mult)

        # diagonal expansions: xdiag[p, j*128+f] = x[128j+p] * (f==p)
        if "diag" not in stages:
            nc.sync.dma_start(out=out_d[b, 0:P, :], in_=v_tiles[0])
            continue
        rdiag = small.tile([P, NJ, P], BF16, tag="rdiag")
        cdiag = small.tile([P, NJ, P], BF16, tag="cdiag")
        vsgdiag = small.tile([P, NJ, P], BF16, tag="vsgdiag")
        for j in range(NJ):
            nc.vector.tensor_scalar(out=rdiag[:, j, :], in0=i128b,
                                    scalar1=r4[:, j:j + 1], scalar2=None,
                                    op0=ALU.mult)
            nc.vector.tensor_scalar(out=cdiag[:, j, :], in0=i128b,
                                    scalar1=c4[:, j:j + 1], scalar2=None,
                                    op0=ALU.mult)
            nc.vector.tensor_scalar(out=vsgdiag[:, j, :], in0=i128b,
                                    scalar1=vs4[:, j:j + 1], scalar2=gam_part,
                                    op0=ALU.mult, op1=ALU.mult)

        # column-sum matmuls -> broadcast rows
        rbc_ps = tps.tile([P, HE], FP32, tag="tp", name=f"rbc_{b}")
        nc.tensor.matmul(rbc_ps, lhsT=ones_kb, rhs=rdiag, start=True, stop=True)
        r_bcast = big.tile([P, HE], FP32, tag="r_bcast")   # r_bcast[p,n] = r[n]
        nc.vector.tensor_copy(out=r_bcast, in_=rbc_ps)

        cbc_ps = tps.tile([P, HE], FP32, tag="tp", name=f"cbc_{b}")
        nc.tensor.matmul(cbc_ps, lhsT=ones_kb, rhs=cdiag, start=True, stop=True)
        c_bcast = big.tile([P, HE], FP32, tag="c_bcast")   # c_bcast[p,n] = c[n]
        nc.vector.tensor_copy(out=c_bcast, in_=cbc_ps)

        vsg_ps = tps.tile([1, HE], FP32, tag="tp", name=f"vsg_{b}")
        nc.tensor.matmul(vsg_ps, lhsT=ones_c1, rhs=vsgdiag, start=True, stop=True)
        vsg_rowb = small.tile([1, HE], BF16, tag="vsg_rowb")  # gamma*vs row
        nc.vector.tensor_copy(out=vsg_rowb, in_=vsg_ps)

        # W[128j+p, n>=128j] = (gamma*r[128j+p]) * G[128j+p, n] * r[n]
        for j in range(NJ):
            nc.vector.scalar_tensor_tensor(out=w_all[:, j, j * P:],
                                           in0=g_tiles[j],
                                           scalar=sc4[:, j:j + 1],
                                           in1=r_bcast[:, j * P:],
                                           op0=ALU.mult, op1=ALU.mult)
        # lower-triangle blocks of W by transposing the upper ones (W = W^T)
        for j in range(1, NJ):
            for jp in range(j):
                wt_ps = tps.tile([P, P], FP32, tag="tp",
                                 name=f"wt_{b}_{j}_{jp}")
                nc.tensor.matmul(wt_ps, lhsT=w_all[:, jp, j * P:(j + 1) * P],
                                 rhs=i128b, start=True, stop=True)
                nc.vector.tensor_copy(out=w_all[:, j, jp * P:(jp + 1) * P],
                                      in_=wt_ps)

        # ---- phase 4: den/tailor computed one quad ahead of part matmuls ----
        if "ph4a" not in stages:
            nc.sync.dma_start(out=out_d[b, 0:P, :], in_=v_tiles[0])
            continue
        den_all = small.tile([P, NLT], FP32, tag="den_all")
        tailor_all = small.tile([P, NLT], FP32, tag="tailor_all")

        use_pemm = TUNE["den_mode"] == "pemm"

        def den_quad(q):
            if use_pemm:
                return
            for ii in range(q * 4, q * 4 + 4):
                scr512 = scr.tile([P, HE], BF16, tag="scr512",
                                  name=f"ttr_{b}_{ii}")
                # den_raw[ii] = sum_n V[l, n] * c[n]
                nc.vector.scalar_tensor_tensor(out=scr512, in0=v_tiles[ii],
                                               scalar=1.0, in1=c_bcast,
                                               op0=ALU.mult, op1=ALU.mult,
                                               accum_out=den_all[:, ii:ii + 1])
            # tailor = 1 / (HE + den_raw), batched per quad
            nc.vector.tensor_scalar(out=den_all[:, q * 4:q * 4 + 4],
                                    in0=den_all[:, q * 4:q * 4 + 4],
                                    scalar1=float(HE), scalar2=None, op0=ALU.add)
            nc.vector.reciprocal(out=tailor_all[:, q * 4:q * 4 + 4],
                                 in_=den_all[:, q * 4:q * 4 + 4])

        den_quad(0)

        # ------------- phase 4b: part matmuls + epilogue -------------
        if "ph4b" not in stages:
            nc.sync.dma_start(out=out_d[b, 0:P, :], in_=v_tiles[0])
            continue
        oq = None
        for i in range(NLT):
            if i % 4 == 0:
                oq = outp.tile([P, 4, HE], FP32, tag="oq", name=f"oq_{b}_{i}")
            if i % 4 == 0 and i // 4 + 1 < NLT // 4:
                den_quad(i // 4 + 1)
            pp = tps.tile([P, HE], FP32, tag="tp", name=f"pp_{b}_{i}")
            dps = None
            if use_pemm:
                dps = tps.tile([P, 1], FP32, tag="dps", name=f"dps_{b}_{i}",
                               bufs=1)
            for j in range(NJ):
                nc.tensor.matmul(pp, lhsT=vt_all[:, j, i * P:(i + 1) * P],
                                 rhs=w_all[:, j, :], start=(j == 0), stop=False)
                if use_pemm:
                    # d[l] += sum_n V[l,n]c[n], same stationary operand
                    nc.tensor.matmul(dps, lhsT=vt_all[:, j, i * P:(i + 1) * P],
                                     rhs=c4b[:, j:j + 1], start=(j == 0),
                                     stop=(j == NJ - 1))
            if use_pemm:
                nc.vector.tensor_scalar(out=den_all[:, i:i + 1], in0=dps,
                                        scalar1=float(HE), scalar2=None,
                                        op0=ALU.add)
                nc.vector.reciprocal(out=tailor_all[:, i:i + 1],
                                     in_=den_all[:, i:i + 1])
            # += gamma * vs[n]  (K=1 matmul broadcasts the row over partitions)
            nc.tensor.matmul(pp, lhsT=ones_r1b, rhs=vsg_rowb, start=False,
                             stop=True)
            if TUNE["ep_mul"] == "vector" or (i % 4) < TUNE["ep_dve_mod"]:
                # fused: out = part*tailor + V on DVE
                nc.vector.scalar_tensor_tensor(out=oq[:, i % 4, :], in0=pp,
                                               scalar=tailor_all[:, i:i + 1],
                                               in1=v_tiles[i],
                                               op0=ALU.mult, op1=ALU.add)
            else:
                # t1 = part * tailor  (ACT, psum->sbuf);  out = t1 + V
                t1 = scr.tile([P, HE], FP32, tag="t1", name=f"t1_{b}_{i}")
                nc.scalar.activation(out=t1, in_=pp, func=AF.Copy,
                                     scale=tailor_all[:, i:i + 1])
                if TUNE["ep_add"] == "gpsimd" and (i % 4) >= TUNE["ep_add_dve_mod"]:
                    nc.gpsimd.tensor_add(out=oq[:, i % 4, :], in0=t1,
                                         in1=v_tiles[i])
                else:
                    nc.vector.tensor_add(out=oq[:, i % 4, :], in0=t1,
                                         in1=v_tiles[i])
            if i % 4 == 3:
                nc.sync.dma_start(
                    out=out_d[b, (i - 3) * P:(i + 1) * P, :].rearrange(
                        "(s p) n -> p s n", p=P),
                    in_=oq)


_PROGRAM_CACHE = {}


def _get_program():
    key = (B_PER, L_FULL)
    if key not in _PROGRAM_CACHE:
        _PROGRAM_CACHE[key] = build_program()
    return _PROGRAM_CACHE[key]


def kernel(queries, keys=None, values=None, attn_mask=None, gamma=None, **kwargs):
    queries = np.ascontiguousarray(np.asarray(queries, dtype=np.float32))
    gamma_np = np.asarray(gamma, dtype=np.float32).reshape(1, 1)
    Bq, Lq, Hq, Eq = queries.shape
    assert (Bq, Lq, Hq, Eq) == (B, L_FULL, H, E)

    qr = queries.reshape(B, L_FULL, HE)
    in_maps = [
        {"q": np.ascontiguousarray(qr[i * B_PER:(i + 1) * B_PER]),
         "gamma": gamma_np}
        for i in range(N_CORES)
    ]
    nc = _get_program()
    res = run_bass_kernel_spmd(nc, in_maps, core_ids=list(range(N_CORES)))
    out = np.concatenate([np.asarray(res.results[i]["out"])
                          for i in range(N_CORES)], axis=0)
    return out.reshape(B, L_FULL, H, E).astype(np.float32)



# revision 12
# speedup vs baseline: 1.7067x; 1.7067x over previous
"""Trainium2 Bass kernel for ChannelLinearAttention (fp8 DoubleRow version).

Math (per batch element, V = queries.reshape(L, HE)):
    G      = V^T V                      [HE, HE]   (Gram over L)
    colsq  = diag(G);  r = 1/sqrt(colsq)
    vs     = sum_l V[l, :]              [HE]
    c      = (vs * r + eps) * r         [HE]
    W      = gamma * G * (r x r)        [HE, HE]
    part   = V @ W + gamma * vs         [L, HE]
    den    = HE + V @ c                 [L]
    out    = V + part / den[:, None]

I/O in fp16 (host casts fp32<->fp16), all matmuls in fp8e4 with
MatmulPerfMode.DoubleRow (two 128-row K-tiles per instruction, 0.5
PE-cycles per output column).  V^T is produced by DR matmuls against a
masked double identity; column sums (vs) by DR matmuls against ones.
PSUM accumulators that take >1 moving window are memset-initialized and
accumulated with start=False so no start=True ever re-marks a live
region (CoreSim zero-region semantics; also avoids relying on HW
whole-bank zeroing).

Sharding: pure data parallel - B=16 batch elements, 2 per NeuronCore.
"""

import numpy as np
from contextlib import ExitStack

import concourse.bass as bass
import concourse.tile as tile
from concourse import mybir
from concourse.bass_utils import run_bass_kernel_spmd
from concourse.masks import make_identity

FP32 = mybir.dt.float32
FP16 = mybir.dt.float16
FP8 = mybir.dt.float8e4
AF = mybir.ActivationFunctionType
ALU = mybir.AluOpType
AX = mybir.AxisListType
DR = mybir.MatmulPerfMode.DoubleRow


class _TC(tile.TileContext):
    """TileContext whose tail drain splits its semaphore waits.

    The walrus CoreV3 codegen on this toolchain rejects a CTRL/NOP-class
    instruction with more than 2 sync waits ("Too many sync wait commands").
    Tile's kernel-tail drain aggregates one wait per live semaphore, which
    exceeds that as soon as a kernel touches >2 queues. Split the waits over
    a chain of SP nops (same engine, in order, before the end barrier) so
    each instruction carries at most 2.
    """

    _MAX_WAITS = 1

    def _drain_and_barrier(self, tick_clock, wait_clock):
        from concourse.vector_clock import ScopedClock

        drain_inst = self.nc.sync.drain()
        wait_clock.add_sem_waits(
            drain_inst.ins, ScopedClock({None: tick_clock.global_clock})
        )
        si = drain_inst.ins.sync_info
        if si is not None and si.on_wait and len(si.on_wait) > self._MAX_WAITS:
            waits = list(si.on_wait)
            chunks = [waits[i:i + self._MAX_WAITS]
                      for i in range(0, len(waits), self._MAX_WAITS)]
            si.on_wait.clear()
            si.on_wait.extend(chunks[0])
            for ch in chunks[1:]:
                nop = self.nc.sync.nop(nofuse=True, hint="tail_drain_split")
                if nop.ins.sync_info is None:
                    nop.ins.sync_info = mybir.SyncInfo(on_wait=[], on_update=[])
                nop.ins.sync_info.on_wait.extend(ch)

        self.nc.all_engine_barrier()
        assert self.sems is not None
        popped = self.nc._tile_sem_poison_stack.pop()
        assert popped is self._sem_poison
        self.nc.clear_and_free_semaphores(list(self.sems.allocated().values()))
        self.nc.all_engine_barrier()

P = 128
B, L_FULL, H, E = 16, 4096, 8, 64
HE = H * E            # 512
N_CORES = 8
B_PER = B // N_CORES  # 2
EPS = 1e-6
GVS_SCALE = 16.0      # gvs stored as gamma*vs/16, restored by 16.0 lhsT


def _split_sync_waits(nc, max_waits=1):
    """Walrus on this toolchain rejects instructions with more than one sync
    wait ("Too many sync wait commands"). Move extra waits onto preceding
    same-engine nops — the engine executes them in order, so semantics are
    preserved."""
    n = 0
    for f in nc.m.functions:
        for blk in f.blocks:
            new_insts = []
            for inst in blk.instructions:
                si = inst.sync_info
                waits = list(si.on_wait) if (si and si.on_wait) else []
                if len(waits) > max_waits:
                    extra, keep = waits[:-max_waits], waits[-max_waits:]
                    for i in range(0, len(extra), max_waits):
                        nop = mybir.InstNoOp(
                            name=f"I-waitsplit-{n}",
                            sync_info=mybir.SyncInfo(
                                on_wait=list(extra[i:i + max_waits]),
                                on_update=[]),
                            bass_nofuse=True,
                            engine=inst.engine,
                        )
                        n += 1
                        nc.register_instruction(nop, overwrite=True)
                        new_insts.append(nop)
                    si.on_wait.clear()
                    si.on_wait.extend(keep)
                new_insts.append(inst)
            blk.instructions[:] = new_insts


ALL_STAGES = frozenset({"ph1", "tail", "ph4"})

# engine placement knobs
TUNE = {
    "cast": "gpsimd",     # engine for fp16->fp8 V casts: gpsimd | vector | scalar
    "vt_copy": "alt",     # V^T psum->sbuf copies: alt (DVE/ACT) | vector | scalar
    "memset": "vector",   # psum zero-init engine: vector | scalar
    "wlt_copy": "scalar",  # W lower-tri copies
    # epilogue routing per chunk half: list of (nh0, nh1) engine modes by i%2
    #   "stt"  = DVE fused (pp*t + V)
    #   "ap"   = ACT scale-copy then Pool add
    "ep_mode": ["stt", "ap"],
    "vq_extra": 3,
    "out_dma": "scalar",   # queue for output DMAs: scalar (ACT HWDGE) | sync
}


def build_program(b_per=B_PER, L=L_FULL, num_devices=N_CORES, stages=ALL_STAGES,
                  repeat=1):
    nc = bass.Bass("TRN2", target_bir_lowering=False, debug=False,
                   num_devices=num_devices)
    q_d = nc.dram_tensor("q", [b_per, L, HE], FP16, kind="ExternalInput").ap()
    gam_d = nc.dram_tensor("gamma", [1, 1], FP32, kind="ExternalInput").ap()
    out_d = nc.dram_tensor("out", [b_per, L, HE], FP16, kind="ExternalOutput").ap()

    with _TC(nc) as tc, ExitStack() as ctx:
        _build(ctx, tc, out_d, q_d, gam_d, b_per, L, stages, repeat)
    _split_sync_waits(nc)
    return nc


def _build(ctx, tc, out_d, q_d, gam_d, b_per, L, stages=ALL_STAGES, repeat=1):
    nc = tc.nc
    NLT = L // P     # 32 l-chunks
    NQ = NLT // 4    # 8 quads
    NJ = HE // P     # 4 n-blocks

    const = ctx.enter_context(tc.tile_pool(name="const", bufs=1))
    vpool = ctx.enter_context(tc.tile_pool(name="vpool", bufs=NQ + TUNE["vq_extra"]))
    v8pool = ctx.enter_context(tc.tile_pool(name="v8pool", bufs=NQ + TUNE["vq_extra"]))
    big = ctx.enter_context(tc.tile_pool(name="big", bufs=1))
    small = ctx.enter_context(tc.tile_pool(name="small", bufs=1))
    scr = ctx.enter_context(tc.tile_pool(name="scr", bufs=2))
    outp = ctx.enter_context(tc.tile_pool(name="outp", bufs=2))
    # PSUM: g tiles 3 banks + vsq ring 1 bank + tp ring 4 banks = 8
    gps = ctx.enter_context(tc.tile_pool(name="gps", bufs=1, space="PSUM"))
    vsp = ctx.enter_context(tc.tile_pool(name="vsp", bufs=1, space="PSUM"))
    tps = ctx.enter_context(tc.tile_pool(name="tps", bufs=4, space="PSUM"))

    def ms_eng():
        return nc.vector if TUNE["memset"] == "vector" else nc.scalar

    # ---------------- constants ----------------
    # id2[p, t, l]: t-masked identity pair: [I|0] on t=0 cols 0:128,
    # [0|I] on t=1 cols 128:256. DR matmul with lhsT=[v8 chunkA, v8 chunkB]
    # transposes both chunks side by side.
    id2 = const.tile([P, 2, 2 * P], FP8)
    nc.gpsimd.memset(id2, 0.0)
    make_identity(nc, id2[:, 0, 0:P], nomemset=True)
    make_identity(nc, id2[:, 1, P:2 * P], nomemset=True)
    # ones over both t (vs column sums). DR LoadWeights requires 128
    # stationary columns (col_grp=0xf) and t-step % 16 == 0, so the ones
    # block is full width; every output row carries the same column sum.
    ones_dr_col = const.tile([P, 2, P], FP8)
    nc.gpsimd.memset(ones_dr_col, 1.0)
    # scaled ones on t=0 only (gvs row broadcast); value undoes GVS_SCALE
    ones_dr_row = const.tile([1, 2, P], FP8)
    nc.gpsimd.memset(ones_dr_row[:, 0, :], GVS_SCALE)
    nc.gpsimd.memset(ones_dr_row[:, 1, :], 0.0)
    i128h = const.tile([P, P], FP16)           # identity (diag mask, fp16)
    make_identity(nc, i128h)
    i128_8 = const.tile([P, P], FP8)           # identity (W transposes, fp8)
    make_identity(nc, i128_8)
    ones_kb = const.tile([P, P], FP16)         # all-ones, r broadcast matmul
    nc.gpsimd.memset(ones_kb, 1.0)
    one_11 = const.tile([1, 1], FP16)
    nc.gpsimd.memset(one_11, 1.0)
    ones_r1f = const.tile([1, P], FP32)
    nc.gpsimd.memset(ones_r1f, 1.0)

    gam_sb = const.tile([1, 1], FP32)
    nc.sync.dma_start(out=gam_sb, in_=gam_d[:, :])
    gam16 = const.tile([1, 1], FP32)           # gamma / GVS_SCALE
    nc.vector.tensor_scalar(out=gam16, in0=gam_sb, scalar1=1.0 / GVS_SCALE,
                            scalar2=None, op0=ALU.mult)
    # broadcast gamma to all 128 partitions: [1,128]^T @ [1,1]
    gam_ps = tps.tile([P, 1], FP32, tag="tp")
    nc.tensor.matmul(gam_ps, lhsT=ones_r1f, rhs=gam_sb, start=True, stop=True)
    gam_part = const.tile([P, 1], FP32)
    nc.scalar.copy(out=gam_part, in_=gam_ps)

    first = True
    for ib, b in enumerate(bb for _ in range(repeat) for bb in range(b_per)):
        first_quad, first = first, False
        # ------------- phase 1: load, cast, Gram, V^T, colsums -------------
        vt8 = big.tile([P, NJ, L], FP8, tag="vt8")     # vt8[p,j,l] = V[l,128j+p]
        w8 = big.tile([P, NJ, HE], FP8, tag="w8")
        # Gram upper blocks; g23 packs rows j=2 (cols 256:512) and j=3
        # (cols 384:512) into one bank.
        g0 = gps.tile([P, HE], FP32, tag="g0", name=f"g0_{ib}")
        g1 = gps.tile([P, HE - P], FP32, tag="g1", name=f"g1_{ib}")
        g23 = gps.tile([P, 384], FP32, tag="g23", name=f"g23_{ib}")
        g_view = [g0, g1, g23[:, 0:256], g23[:, 256:384]]
        vs_ps = vsp.tile([P, HE], FP32, tag="vsq", name=f"vs_{ib}")
        for t_ in (g0, g1, g23):
            ms_eng().memset(t_, 0.0)
        ms_eng().memset(vs_ps, 0.0)

        v_tiles, v8_tiles = [], []
        for kq in range(NQ):
            vq = vpool.tile([P, 4, HE], FP16, tag="v", name=f"v_{ib}_{kq}")
            v8q = v8pool.tile([P, 4, HE], FP8, tag="v8", name=f"v8_{ib}_{kq}")
            cast_eng = {"gpsimd": nc.gpsimd, "vector": nc.vector,
                        "scalar": nc.scalar}[TUNE["cast"]]

            def _cast(dst, src):
                if cast_eng is nc.scalar:
                    nc.scalar.copy(out=dst, in_=src)
                else:
                    cast_eng.tensor_copy(out=dst, in_=src)

            if first_quad and kq == 0:
                # split first load/cast so PE can start earlier
                for hh in range(2):
                    nc.sync.dma_start(
                        out=vq[:, 2 * hh:2 * hh + 2, :],
                        in_=q_d[b, (4 * kq + 2 * hh) * P:(4 * kq + 2 * hh + 2) * P, :]
                        .rearrange("(s p) n -> p s n", p=P))
                    _cast(v8q[:, 2 * hh:2 * hh + 2, :], vq[:, 2 * hh:2 * hh + 2, :])
            else:
                nc.sync.dma_start(
                    out=vq,
                    in_=q_d[b, kq * 4 * P:(kq + 1) * 4 * P, :].rearrange(
                        "(s p) n -> p s n", p=P))
                _cast(v8q, vq)
            for s in range(4):
                v_tiles.append(vq[:, s, :])
                v8_tiles.append(v8q[:, s, :])

            if "ph1" not in stages:
                continue
            for h in range(2):
                pair = v8q[:, 2 * h:2 * h + 2, :]
                # Gram upper-block windows (<=512 moving elems each)
                for j, w0, w1 in ((0, 0, 256), (0, 256, 512),
                                  (1, 128, 384), (1, 384, 512),
                                  (2, 256, 512), (3, 384, 512)):
                    nc.tensor.matmul(
                        g_view[j][:, w0 - j * P:w1 - j * P],
                        lhsT=pair[:, :, j * P:(j + 1) * P],
                        rhs=pair[:, :, w0:w1],
                        start=False, stop=(kq == NQ - 1 and h == 1),
                        perf_mode=DR, skip_group_check=True)
                # vs column sums
                for nh in range(2):
                    nc.tensor.matmul(
                        vs_ps[:, nh * 256:(nh + 1) * 256],
                        lhsT=ones_dr_col,
                        rhs=pair[:, :, nh * 256:(nh + 1) * 256],
                        start=False, stop=(kq == NQ - 1 and h == 1),
                        perf_mode=DR, skip_group_check=True)
                # transposes: one DR matmul flips both chunks of the pair
                for pr in range(2):
                    t = tps.tile([P, 2, 2 * P], FP32, tag="tp",
                                 name=f"t_{ib}_{kq}_{h}_{pr}")
                    for jj in range(2):
                        j = 2 * pr + jj
                        nc.tensor.matmul(t[:, jj, :],
                                         lhsT=pair[:, :, j * P:(j + 1) * P],
                                         rhs=id2, start=True, stop=True,
                                         perf_mode=DR)
                    lbase = (4 * kq + 2 * h) * P
                    mode = TUNE["vt_copy"]
                    use_act = (mode == "scalar" or (mode == "alt" and pr == 1))
                    dst = vt8[:, 2 * pr:2 * pr + 2, lbase:lbase + 2 * P]
                    if use_act:
                        nc.scalar.copy(out=dst, in_=t)
                    else:
                        nc.vector.tensor_copy(out=dst, in_=t)

        # ------------- phase 2/3: tail math -------------
        if "tail" not in stages:
            nc.sync.dma_start(out=out_d[b, 0:P, :], in_=v_tiles[0])
            continue
        # vs row -> per-partition vs4[p, j] = vs[128j+p]
        vs_sb = small.tile([1, HE], FP16, tag="vs_sb")
        nc.scalar.copy(out=vs_sb, in_=vs_ps[0:1, :])
        vs4_ps = tps.tile([P, NJ], FP32, tag="tp", name=f"vs4_{ib}")
        for j in range(NJ):
            nc.tensor.matmul(vs4_ps[:, j:j + 1],
                             lhsT=vs_sb[:, j * P:(j + 1) * P], rhs=one_11,
                             start=True, stop=True)
        vs4 = small.tile([P, NJ], FP32, tag="vs4")
        nc.vector.tensor_copy(out=vs4, in_=vs4_ps)
        # gvs row (gamma*vs/16) in fp8, t1 lane zeroed
        gvs8 = small.tile([1, 2, 2, 256], FP8, tag="gvs8")
        nc.gpsimd.memset(gvs8, 0.0)
        for nh in range(2):
            nc.scalar.activation(out=gvs8[:, nh, 0, :],
                                 in_=vs_ps[0:1, nh * 256:(nh + 1) * 256],
                                 func=AF.Copy, scale=gam16)

        # colsq[128j+p] = G[128j+p, 128j+p]: masked row-sum of G's diag block
        colsq4 = small.tile([P, NJ], FP32, tag="colsq4")
        dscr = scr.tile([P, NJ, P], FP32, tag="dscr", name=f"dscr_{ib}")
        for j in range(NJ):
            nc.vector.scalar_tensor_tensor(out=dscr[:, j, :],
                                           in0=g_view[j][:, 0:P],
                                           scalar=1.0, in1=i128h,
                                           op0=ALU.mult, op1=ALU.mult,
                                           accum_out=colsq4[:, j:j + 1])
        norm4 = small.tile([P, NJ], FP32, tag="norm4")
        nc.scalar.sqrt(out=norm4, in_=colsq4)
        r4 = small.tile([P, NJ], FP32, tag="r4")
        nc.vector.reciprocal(out=r4, in_=norm4)
        # c = (vs*r + eps) * r; fp8 copy for the den matmuls
        c4 = small.tile([P, NJ], FP32, tag="c4")
        nc.vector.tensor_mul(out=c4, in0=vs4, in1=r4)
        nc.vector.tensor_scalar(out=c4, in0=c4, scalar1=EPS, scalar2=None,
                                op0=ALU.add)
        nc.vector.tensor_mul(out=c4, in0=c4, in1=r4)
        c8 = small.tile([P, NJ, 1], FP8, tag="c8")
        nc.vector.tensor_copy(out=c8[:, :, 0], in_=c4)
        # sc4 = gamma * r  (per-partition scale for W rows)
        sc4 = small.tile([P, NJ], FP32, tag="sc4")
        nc.vector.tensor_scalar(out=sc4, in0=r4, scalar1=gam_part, scalar2=None,
                                op0=ALU.mult)

        # r broadcast row: rdiag = r*I per block, ones^T @ rdiag
        rdiag = small.tile([P, NJ, P], FP16, tag="rdiag")
        for j in range(NJ):
            nc.vector.tensor_scalar(out=rdiag[:, j, :], in0=i128h,
                                    scalar1=r4[:, j:j + 1], scalar2=None,
                                    op0=ALU.mult)
        rbc_ps = tps.tile([P, HE], FP32, tag="tp", name=f"rbc_{ib}")
        nc.tensor.matmul(rbc_ps, lhsT=ones_kb, rhs=rdiag, start=True, stop=True)
        r_bcast = small.tile([P, HE], FP32, tag="r_bcast")
        nc.vector.tensor_copy(out=r_bcast, in_=rbc_ps)

        # W upper blocks: W[128j+p, n>=128j] = sc[128j+p] * G[...] * r[n]
        for j in range(NJ):
            nc.vector.scalar_tensor_tensor(out=w8[:, j, j * P:],
                                           in0=g_view[j],
                                           scalar=sc4[:, j:j + 1],
                                           in1=r_bcast[:, j * P:],
                                           op0=ALU.mult, op1=ALU.mult)
        # W lower blocks by transposing the upper ones (W = W^T)
        for j in range(1, NJ):
            for jp in range(j):
                wt_ps = tps.tile([P, P], FP32, tag="tp",
                                 name=f"wt_{ib}_{j}_{jp}")
                nc.tensor.matmul(wt_ps, lhsT=w8[:, jp, j * P:(j + 1) * P],
                                 rhs=i128_8, start=True, stop=True)
                wdst = w8[:, j, jp * P:(jp + 1) * P]
                if TUNE["wlt_copy"] == "scalar":
                    nc.scalar.copy(out=wdst, in_=wt_ps)
                else:
                    nc.vector.tensor_copy(out=wdst, in_=wt_ps)

        # ---- phase 4: part/den matmuls + epilogue ----
        if "ph4" not in stages:
            nc.sync.dma_start(out=out_d[b, 0:P, :], in_=v_tiles[0])
            continue
        den_all = small.tile([P, NLT], FP32, tag="den_all")
        tailor_all = small.tile([P, NLT], FP32, tag="tailor_all")

        oq = None
        dq = None
        for i in range(NLT):
            if i % 4 == 0:
                oq = outp.tile([P, 4, HE], FP16, tag="oq", name=f"oq_{ib}_{i}")
                dq = vsp.tile([P, 4], FP32, tag="vsq", name=f"dq_{ib}_{i}")
            pps = [tps.tile([P, 256], FP32, tag="tp", name=f"pp{nh}_{ib}_{i}")
                   for nh in range(2)]
            for q in range(2):
                lhsT = vt8[:, 2 * q:2 * q + 2, i * P:(i + 1) * P]
                for nh in range(2):
                    nc.tensor.matmul(pps[nh],
                                     lhsT=lhsT,
                                     rhs=w8[:, 2 * q:2 * q + 2,
                                            nh * 256:(nh + 1) * 256],
                                     start=(q == 0), stop=False,
                                     perf_mode=DR, skip_group_check=True)
                # den: d[l] += sum_n V[l,n] c[n], same stationary operand
                nc.tensor.matmul(dq[:, i % 4:i % 4 + 1], lhsT=lhsT,
                                 rhs=c8[:, 2 * q:2 * q + 2, :],
                                 start=(q == 0), stop=(q == 1),
                                 perf_mode=DR, skip_group_check=True)
            # += gamma*vs[n] row (DR rank-1 broadcast closes the pp groups)
            for nh in range(2):
                nc.tensor.matmul(pps[nh], lhsT=ones_dr_row, rhs=gvs8[:, nh],
                                 start=False, stop=True,
                                 perf_mode=DR, skip_group_check=True)
            # tailor = 1 / (HE + den)
            nc.vector.tensor_scalar(out=den_all[:, i:i + 1],
                                    in0=dq[:, i % 4:i % 4 + 1],
                                    scalar1=float(HE), scalar2=None,
                                    op0=ALU.add)
            nc.vector.reciprocal(out=tailor_all[:, i:i + 1],
                                 in_=den_all[:, i:i + 1])
            # epilogue: out = pp*tailor + V, per 256-half
            ep = TUNE["ep_mode"][i % len(TUNE["ep_mode"])]
            t1 = (scr.tile([P, 2, 256], FP16, tag="t1", name=f"t1_{ib}_{i}")
                  if ep == "ap" else None)
            for nh in range(2):
                dst = oq[:, i % 4, nh * 256:(nh + 1) * 256]
                vsrc = v_tiles[i][:, nh * 256:(nh + 1) * 256]
                if ep == "stt":
                    nc.vector.scalar_tensor_tensor(
                        out=dst, in0=pps[nh],
                        scalar=tailor_all[:, i:i + 1], in1=vsrc,
                        op0=ALU.mult, op1=ALU.add)
                else:
                    nc.scalar.activation(out=t1[:, nh, :], in_=pps[nh],
                                         func=AF.Copy,
                                         scale=tailor_all[:, i:i + 1])
                    nc.gpsimd.tensor_add(out=dst, in0=t1[:, nh, :], in1=vsrc)
            if i % 4 == 3:
                out_eng = nc.scalar if TUNE["out_dma"] == "scalar" else nc.sync
                out_eng.dma_start(
                    out=out_d[b, (i - 3) * P:(i + 1) * P, :].rearrange(
                        "(s p) n -> p s n", p=P),
                    in_=oq)


_PROGRAM_CACHE = {}


def _get_program():
    key = (B_PER, L_FULL)
    if key not in _PROGRAM_CACHE:
        _PROGRAM_CACHE[key] = build_program()
    return _PROGRAM_CACHE[key]


def kernel(queries, keys=None, values=None, attn_mask=None, gamma=None, **kwargs):
    queries = np.asarray(queries)
    gamma_np = np.asarray(gamma, dtype=np.float32).reshape(1, 1)
    Bq, Lq, Hq, Eq = queries.shape
    assert (Bq, Lq, Hq, Eq) == (B, L_FULL, H, E)

    q16 = np.ascontiguousarray(
        queries.reshape(B, L_FULL, HE).astype(np.float16))
    in_maps = [
        {"q": q16[i * B_PER:(i + 1) * B_PER], "gamma": gamma_np}
        for i in range(N_CORES)
    ]
    nc = _get_program()
    res = run_bass_kernel_spmd(nc, in_maps, core_ids=list(range(N_CORES)))
    out = np.concatenate([np.asarray(res.results[i]["out"])
                          for i in range(N_CORES)], axis=0)
    return out.reshape(B, L_FULL, H, E).astype(np.float32)
